# revision 28
# baseline (speedup 1.0000x reference)
"""CACE GNN message-passing kernel for 8 trn2 NeuronCores (v3).

Node-parallel sharding (625 nodes/core), edges sorted by receiver and packed
into <=128-edge x <=16-node chunks. Structure exploited: within a segment all
edges share the receiver node, so A[n,r,m,(cs,cr)] factorizes as
Atilde[n,r,m,cs] * emb_recv[n,cr]; the nu=2..4 symmetrization runs on 3
channels instead of 9 and the result is expanded by emb_recv^nu at the end.

Per core:
  1. host ships per-edge unit vector, sender embedding, and
     lhsT = onehot(node,16) x radial(8)  [128e, 128] f32 (DMA-only, overlaps
     compute); device builds rhs = (1,V,S9,T27) x emb_send [128e, 120] f32,
  2. one f32 matmul per chunk -> PSUM; ACT/Pool copies -> Atilde slab (bf16),
  3. dense bf16 symmetrization (ordered index sets, strided tensor ops with
     <=3 free dims, tree adds) across DVE/Pool/ACT -> 11 features x 3 cs,
  4. f32 expansion by emb_recv^nu(f), f-major layout -> contiguous DMAs out.
"""
import math
import functools
import numpy as np

# ---------------- problem constants ----------------------------------------
N_NODES, N_EDGES = 5000, 50000
N_RBF = 8
CUTOFF = 5.5
EPS = 1e-9
ZS = [1, 6, 7, 8]
N_CORES = 8
PER = N_NODES // N_CORES          # 625 nodes per core
NT = 16                           # nodes per chunk
N_CH = 53                         # chunks per core (padded; max measured 53)
P = 128                           # edges per chunk (partitions)
NQ = NT * N_RBF                   # 128 = lhsT free dim
MB = 40                           # ordered monomial planes: 1 + 3 + 9 + 27
NF = 11
CH = N_CH


# ---------------- device kernel build --------------------------------------
@functools.lru_cache(maxsize=2)
def _build_nc(debug=False):
    import concourse.bass as bass
    import concourse.bacc as bacc
    import concourse.mybir as mybir
    from concourse.tile import TileContext

    f32 = mybir.dt.float32
    bf16 = mybir.dt.bfloat16
    MUL = mybir.AluOpType.mult
    ADD = mybir.AluOpType.add
    SUB = mybir.AluOpType.subtract
    ACT = mybir.ActivationFunctionType

    nc = bacc.Bacc("TRN2", target_bir_lowering=False, debug=False,
                   num_devices=N_CORES)
    ed_d = nc.dram_tensor("ed", [P, CH * 6], f32, kind="ExternalInput")
    lh_d = nc.dram_tensor("lh", [P, CH * NQ], f32, kind="ExternalInput")
    ebr_d = nc.dram_tensor("ebr", [P, CH * 3], f32, kind="ExternalInput")
    out_d = nc.dram_tensor("out", [P, NF * CH * 9], f32,
                           kind="ExternalOutput")
    dbg = {}
    if debug:
        for nm, w in [("rhs", CH * MB * 3), ("lhsT", CH * NQ)]:
            dbg[nm] = nc.dram_tensor("dbg_" + nm, [P, w], f32,
                                     kind="ExternalOutput")
        for nm, w in [("A", CH * MB * 3), ("Q", CH * 39 * 3),
                      ("Ft", CH * NF * 3)]:
            dbg[nm] = nc.dram_tensor("dbg_" + nm, [P, w], mybir.dt.bfloat16,
                                     kind="ExternalOutput")

    HALF = 27                      # chunk split for pipelined rhs/matmuls

    with TileContext(nc) as tc:
        with (
            tc.tile_pool(name="io", bufs=1) as io,
            tc.tile_pool(name="work", bufs=1) as wk,
            tc.tile_pool(name="psum", bufs=4, space="PSUM") as pp,
        ):
            ed = io.tile([P, CH * 6], f32)
            lhsT = io.tile([P, CH * NQ], f32)
            ebr = io.tile([P, CH * 3], f32)
            nc.sync.dma_start(out=ed[:, :], in_=ed_d[:, :])
            nc.sync.dma_start(out=lhsT[:, :HALF * NQ],
                              in_=lh_d[:, :HALF * NQ])
            nc.sync.dma_start(out=lhsT[:, HALF * NQ:],
                              in_=lh_d[:, HALF * NQ:])
            nc.sync.dma_start(out=ebr[:, :], in_=ebr_d[:, :])

            edv = ed[:, :].rearrange("p (ch t) -> p ch t", t=6)

            # ---- rhs: ordered basis x embS, cascaded (f32), two halves ----
            S9 = wk.tile([P, CH * 9], f32)
            rhs = wk.tile([P, CH * MB * 3], f32)
            for lo, hi in ((0, HALF), (HALF, CH)):
                w = hi - lo
                unitv = edv[:, lo:hi, 0:3]
                embS = edv[:, lo:hi, 3:6]
                s9v = S9[:, :].rearrange("p (ch a b) -> p ch a b", a=3,
                                         b=3)[:, lo:hi]
                nc.vector.tensor_tensor(
                    out=s9v,
                    in0=unitv.unsqueeze(3).to_broadcast([P, w, 3, 3]),
                    in1=unitv.unsqueeze(2).to_broadcast([P, w, 3, 3]),
                    op=MUL)
                rv = rhs[:, :].rearrange("p (ch m c) -> p ch m c", m=MB,
                                         c=3)[:, lo:hi]
                nc.scalar.copy(out=rv[:, :, 0, :], in_=embS)
                nc.vector.tensor_tensor(
                    out=rv[:, :, 1:4, :],
                    in0=unitv.unsqueeze(3).to_broadcast([P, w, 3, 3]),
                    in1=embS.unsqueeze(2).to_broadcast([P, w, 3, 3]),
                    op=MUL)
                ve9 = rv[:, :, 1:4, :].rearrange("p ch b c -> p ch (b c)")
                nc.vector.tensor_tensor(
                    out=rv[:, :, 4:13, :].rearrange(
                        "p ch (a b) c -> p ch a (b c)", a=3),
                    in0=unitv.unsqueeze(3).to_broadcast([P, w, 3, 9]),
                    in1=ve9.unsqueeze(2).to_broadcast([P, w, 3, 9]),
                    op=MUL)
                nc.vector.tensor_tensor(
                    out=rv[:, :, 13:40, :].rearrange(
                        "p ch (ab cc) c -> p ch ab (cc c)", ab=9),
                    in0=s9v.rearrange("p ch a b -> p ch (a b)")
                        .unsqueeze(3).to_broadcast([P, w, 9, 9]),
                    in1=ve9.unsqueeze(2).to_broadcast([P, w, 9, 9]),
                    op=MUL)

            # ---- matmuls (f32) + ACT/Pool psum->sbuf copies (bf16 A) ----
            A = wk.tile([P, CH * MB * 3], bf16)
            Av = A[:, :].rearrange("p (ch m c) -> p ch m c", m=MB, c=3)
            rflat = rhs[:, :].rearrange("p (ch f) -> p ch f", f=MB * 3)
            lflat = lhsT[:, :].rearrange("p (ch q) -> p ch q", q=NQ)
            Aflat = A[:, :].rearrange("p (ch f) -> p ch f", f=MB * 3)
            GW = 8
            n_grp = (CH + GW - 1) // GW
            with nc.allow_low_precision(reason="bf16 A slab"):
                for g in range(n_grp):
                    c0 = g * GW
                    c1 = min(CH, c0 + GW)
                    # 2 PSUM banks; 4 chunks per bank (120*4 f32 <= 512)
                    pt = pp.tile([P, 1024], f32)
                    for ch in range(c0, c1):
                        k = ch - c0
                        col = (k // 4) * 512 + (k % 4) * 120
                        nc.tensor.matmul(
                            out=pt[:, col:col + 120],
                            lhsT=lflat[:, ch, :], rhs=rflat[:, ch, :],
                            start=True, stop=True)
                    nb = c1 - c0
                    ptb = pt[:, :].rearrange("p (b x) -> p b x", b=2)
                    ob = Aflat[:, c0:c1, :].rearrange(
                        "p ch f -> p (ch f)").rearrange(
                        "p (b x) -> p b x", b=2) if nb == 8 else None
                    if nb == 8:
                        if g % 2 == 0:
                            nc.scalar.copy(out=ob, in_=ptb[:, :, :480])
                        else:
                            nc.vector.tensor_copy(out=ob,
                                                  in_=ptb[:, :, :480])
                    else:
                        for bi in range((nb + 3) // 4):
                            w = min(4, nb - bi * 4) * 120
                            nc.scalar.copy(
                                out=Aflat[:, c0 + bi * 4:
                                          c0 + bi * 4 + w // 120, :]
                                .rearrange("p ch f -> p (ch f)"),
                                in_=ptb[:, bi, :w])

            # ---- symmetrization (bf16, <=3 free dims per AP) ----
            def APL(m0, m1=None):
                m1 = m0 + 1 if m1 is None else m1
                return Av[:, :, m0:m1, :].rearrange("p ch m c -> p ch (m c)")

            with nc.allow_low_precision(reason="bf16 feature pipeline"):
                Q = wk.tile([P, CH * 39 * 3], bf16)
                a139 = Av[:, :, 1:40, :].rearrange("p ch m c -> p ch (m c)")
                qf = Q[:, :].rearrange("p (ch m c) -> p ch (m c)", m=39, c=3)
                nc.vector.tensor_tensor(out=qf, in0=a139, in1=a139, op=MUL)
                Qv = Q[:, :].rearrange("p (ch m c) -> p ch m c", m=39, c=3)

                def QPL(m0, m1=None):
                    m1 = m0 + 1 if m1 is None else m1
                    return Qv[:, :, m0:m1, :].rearrange(
                        "p ch m c -> p ch (m c)")

                Ft = wk.tile([P, CH * NF * 3], bf16)
                Fv = Ft[:, :].rearrange("p (ch f c) -> p ch f c", f=NF, c=3)

                def FPL(f):
                    return Fv[:, :, f, :]

                def c3(dst, srcs, eng=None):
                    e = eng or nc.vector
                    e.tensor_tensor(out=dst, in0=srcs[0], in1=srcs[1], op=ADD)
                    e.tensor_tensor(out=dst, in0=dst, in1=srcs[2], op=ADD)

                # tz_abc = T_abc * S_ab : per-cc products (Pool)
                tz = wk.tile([P, CH * 81], bf16)
                tzv = tz[:, :].rearrange("p (ch ab cc c) -> p ch ab cc c",
                                         ab=9, cc=3, c=3)
                Tv = Av[:, :, 13:40, :].rearrange(
                    "p ch (ab cc) c -> p ch ab cc c", ab=9, cc=3)
                spl9 = Av[:, :, 4:13, :].rearrange(
                    "p ch (ab) c -> p ch ab c", ab=9)
                for cc in range(3):
                    nc.gpsimd.tensor_tensor(
                        out=tzv[:, :, :, cc, :], in0=Tv[:, :, :, cc, :],
                        in1=spl9, op=MUL)

                # S2 products: per-(a,k) (Pool)
                ss = wk.tile([P, CH * 81], bf16)
                ss4 = ss[:, :].rearrange("p (ch ak b c) -> p ch ak b c",
                                         ak=9, b=3, c=3)
                for a in range(3):
                    for k in range(3):
                        nc.gpsimd.tensor_tensor(
                            out=ss4[:, :, a * 3 + k, :, :],
                            in0=Av[:, :, 4 + 3 * a + k, :].unsqueeze(2)
                                .to_broadcast([P, CH, 3, 3]),
                            in1=Av[:, :, 4 + 3 * k:7 + 3 * k, :],
                            op=MUL)

                # F0 = A plane 0
                nc.scalar.copy(out=FPL(0), in_=APL(0))
                # nu2_1
                c3(FPL(1), [QPL(i) for i in range(3)])
                # nu2_2
                t9 = wk.tile([P, CH * 9], bf16)
                t9v = t9[:, :].rearrange("p (ch x) -> p ch x", x=9)
                c3(t9v, [QPL(3 + 3 * i, 6 + 3 * i) for i in range(3)])
                t3 = t9[:, :].rearrange("p (ch b c) -> p ch b c", b=3, c=3)
                c3(FPL(2), [t3[:, :, i, :] for i in range(3)])
                # nu2_3
                t27 = wk.tile([P, CH * 27], bf16)
                t27v = t27[:, :].rearrange("p (ch x) -> p ch x", x=27)
                c3(t27v, [QPL(12 + 9 * i, 21 + 9 * i) for i in range(3)])
                t27b = t27[:, :].rearrange("p (ch b cc) -> p ch b cc", b=3,
                                           cc=9)
                c3(t9v, [t27b[:, :, i, :] for i in range(3)])
                c3(FPL(3), [t3[:, :, i, :] for i in range(3)])

                # u_a = sum_b S_ab V_b
                u9 = wk.tile([P, CH * 27], bf16)
                u9v = u9[:, :].rearrange("p (ch a bc) -> p ch a bc", a=3,
                                         bc=9)
                vpl = APL(1, 4)
                for a in range(3):
                    nc.vector.tensor_tensor(
                        out=u9v[:, :, a, :], in0=APL(4 + 3 * a, 7 + 3 * a),
                        in1=vpl, op=MUL)
                u_ = wk.tile([P, CH * 9], bf16)
                uv9 = u_[:, :].rearrange("p (ch x) -> p ch x", x=9)
                u9b = u9[:, :].rearrange("p (ch a b c) -> p ch a b c", a=3,
                                         b=3, c=3)
                uvb = u_[:, :].rearrange("p (ch a c) -> p ch a c", a=3, c=3)
                nc.vector.tensor_tensor(out=uvb, in0=u9b[:, :, :, 0, :],
                                        in1=u9b[:, :, :, 1, :], op=ADD)
                nc.vector.tensor_tensor(out=uvb, in0=uvb,
                                        in1=u9b[:, :, :, 2, :], op=ADD)

                # M_cd (c<=d) products + trees (DVE)
                mprod = wk.tile([P, CH * 9 * 6 * 3], bf16)
                mpv = mprod[:, :].rearrange("p (ch ab k c) -> p ch ab k c",
                                            ab=9, k=6, c=3)
                cdpairs = [(0, 0), (0, 1), (0, 2), (1, 1), (1, 2), (2, 2)]
                for ki, (c, dd) in enumerate(cdpairs):
                    nc.vector.tensor_tensor(
                        out=mpv[:, :, :, ki, :],
                        in0=Tv[:, :, :, c, :], in1=Tv[:, :, :, dd, :],
                        op=MUL)
                mpa = mprod[:, :].rearrange("p (ch a rest) -> p ch a rest",
                                            a=3, rest=54)
                m54 = wk.tile([P, CH * 54], bf16)
                m54v = m54[:, :].rearrange("p (ch x) -> p ch x", x=54)
                c3(m54v, [mpa[:, :, i, :] for i in range(3)])
                m54b = m54[:, :].rearrange("p (ch b y) -> p ch b y", b=3,
                                           y=18)
                mm = wk.tile([P, CH * 18], bf16)
                mmv18 = mm[:, :].rearrange("p (ch x) -> p ch x", x=18)
                c3(mmv18, [m54b[:, :, i, :] for i in range(3)])
                mmv = mm[:, :].rearrange("p (ch k c) -> p ch k c", k=6, c=3)

                # z trees (tz from Pool)
                tza = tz[:, :].rearrange("p (ch a rest) -> p ch a rest", a=3,
                                         rest=27)
                z9 = wk.tile([P, CH * 27], bf16)
                z9a = z9[:, :].rearrange("p (ch x) -> p ch x", x=27)
                c3(z9a, [tza[:, :, i, :] for i in range(3)])
                z9b = z9[:, :].rearrange("p (ch b y) -> p ch b y", b=3, y=9)
                z_ = wk.tile([P, CH * 9], bf16)
                zv9 = z_[:, :].rearrange("p (ch x) -> p ch x", x=9)
                c3(zv9, [z9b[:, :, i, :] for i in range(3)])

                # P2 products (DVE) + tree
                pv = wk.tile([P, CH * 81], bf16)
                pvv = pv[:, :].rearrange("p (ch a x) -> p ch a x", a=3, x=27)
                pv4 = pv[:, :].rearrange("p (ch a bc c) -> p ch a bc c", a=3,
                                         bc=9, c=3)
                for a in range(3):
                    nc.vector.tensor_tensor(
                        out=pv4[:, :, a, :, :],
                        in0=Av[:, :, 13 + 9 * a:22 + 9 * a, :],
                        in1=Av[:, :, 1 + a, :].unsqueeze(2).to_broadcast(
                            [P, CH, 9, 3]),
                        op=MUL)
                p2 = wk.tile([P, CH * 27], bf16)
                p2v = p2[:, :].rearrange("p (ch x) -> p ch x", x=27)
                c3(p2v, [pvv[:, :, i, :] for i in range(3)])

                # S2 trees (ss from Pool) + trS3
                s2 = wk.tile([P, CH * 27], bf16)
                s2v = s2[:, :].rearrange("p (ch x) -> p ch x", x=27)
                ssk = ss[:, :].rearrange("p (ch a k bc) -> p ch a k bc", a=3,
                                         k=3, bc=9)
                s2m = s2[:, :].rearrange("p (ch a bc) -> p ch a bc", a=3,
                                         bc=9)
                nc.vector.tensor_tensor(out=s2m, in0=ssk[:, :, :, 0, :],
                                        in1=ssk[:, :, :, 1, :], op=ADD)
                nc.vector.tensor_tensor(out=s2m, in0=s2m,
                                        in1=ssk[:, :, :, 2, :], op=ADD)
                w9 = wk.tile([P, CH * 27], bf16)
                w9v = w9[:, :].rearrange("p (ch x) -> p ch x", x=27)
                nc.vector.tensor_tensor(out=w9v, in0=s2v, in1=APL(4, 13),
                                        op=MUL)
                w9a = w9[:, :].rearrange("p (ch a y) -> p ch a y", a=3, y=9)
                c3(t9v, [w9a[:, :, i, :] for i in range(3)])
                c3(FPL(4), [t3[:, :, i, :] for i in range(3)])

                # nu3_2 = 2*sum_{c<=d} M_cd S_cd - sum_c M_cc S_cc
                q6 = wk.tile([P, CH * 18], bf16)
                q6v = q6[:, :].rearrange("p (ch k c) -> p ch k c", k=6, c=3)
                nc.vector.tensor_tensor(
                    out=q6v[:, :, 0:3, :].rearrange("p ch k c -> p ch (k c)"),
                    in0=mmv[:, :, 0:3, :].rearrange("p ch k c -> p ch (k c)"),
                    in1=APL(4, 7), op=MUL)
                nc.vector.tensor_tensor(
                    out=q6v[:, :, 3:5, :].rearrange("p ch k c -> p ch (k c)"),
                    in0=mmv[:, :, 3:5, :].rearrange("p ch k c -> p ch (k c)"),
                    in1=APL(8, 10), op=MUL)
                nc.vector.tensor_tensor(
                    out=q6v[:, :, 5, :], in0=mmv[:, :, 5, :], in1=APL(12),
                    op=MUL)
                sall = wk.tile([P, CH * 3], bf16)
                sallv = sall[:, :].rearrange("p (ch c) -> p ch c", c=3)
                t2s = wk.tile([P, CH * 6], bf16)
                t2sv = t2s[:, :].rearrange("p (ch x) -> p ch x", x=6)
                q6f = q6[:, :].rearrange("p (ch x) -> p ch x", x=18)
                nc.vector.tensor_tensor(out=t2sv, in0=q6f[:, :, 0:6],
                                        in1=q6f[:, :, 6:12], op=ADD)
                t2sk = t2s[:, :].rearrange("p (ch k c) -> p ch k c", k=2, c=3)
                nc.vector.tensor_tensor(out=sallv, in0=t2sk[:, :, 0, :],
                                        in1=t2sk[:, :, 1, :], op=ADD)
                nc.vector.tensor_tensor(out=sallv, in0=sallv,
                                        in1=q6v[:, :, 4, :], op=ADD)
                nc.vector.tensor_tensor(out=sallv, in0=sallv,
                                        in1=q6v[:, :, 5, :], op=ADD)
                sdia = wk.tile([P, CH * 3], bf16)
                sdiav = sdia[:, :].rearrange("p (ch c) -> p ch c", c=3)
                nc.vector.tensor_tensor(out=sdiav, in0=q6v[:, :, 0, :],
                                        in1=q6v[:, :, 3, :], op=ADD)
                nc.vector.tensor_tensor(out=sdiav, in0=sdiav,
                                        in1=q6v[:, :, 5, :], op=ADD)
                nc.vector.scalar_tensor_tensor(
                    out=FPL(5), in0=sallv, scalar=2.0, in1=sdiav,
                    op0=MUL, op1=SUB)

                # nu4 features (squares on ACT, cross product on DVE)
                uu3 = wk.tile([P, CH * 9], bf16)
                uu3b = uu3[:, :].rearrange("p (ch a c) -> p ch a c", a=3, c=3)
                nc.scalar.activation(out=uu3[:, :], in_=u_[:, :],
                                     func=ACT.Square)
                c3(FPL(6), [uu3b[:, :, i, :] for i in range(3)])
                uz3 = wk.tile([P, CH * 9], bf16)
                uz3v = uz3[:, :].rearrange("p (ch x) -> p ch x", x=9)
                uz3b = uz3[:, :].rearrange("p (ch a c) -> p ch a c", a=3, c=3)
                nc.vector.tensor_tensor(out=uz3v, in0=uv9, in1=zv9, op=MUL)
                c3(FPL(7), [uz3b[:, :, i, :] for i in range(3)])
                nc.scalar.copy(out=FPL(9), in_=FPL(7))
                zz3 = wk.tile([P, CH * 9], bf16)
                zz3b = zz3[:, :].rearrange("p (ch a c) -> p ch a c", a=3, c=3)
                nc.scalar.activation(out=zz3[:, :], in_=z_[:, :],
                                     func=ACT.Square)
                c3(FPL(10), [zz3b[:, :, i, :] for i in range(3)])
                pp9 = wk.tile([P, CH * 27], bf16)
                nc.scalar.activation(out=pp9[:, :], in_=p2[:, :],
                                     func=ACT.Square)
                pp9b = pp9[:, :].rearrange("p (ch b cc) -> p ch b cc", b=3,
                                           cc=9)
                c3(t9v, [pp9b[:, :, i, :] for i in range(3)])
                c3(FPL(8), [t3[:, :, i, :] for i in range(3)])

            # ---- expansion by emb_recv^nu(f) (f32), f-major output ----
            ebv = ebr[:, :].rearrange("p (ch c) -> p ch c", c=3)
            e2 = wk.tile([P, CH * 3], f32)
            e2v = e2[:, :].rearrange("p (ch c) -> p ch c", c=3)
            nc.vector.tensor_tensor(out=e2v, in0=ebv, in1=ebv, op=MUL)
            e3 = wk.tile([P, CH * 3], f32)
            e3v = e3[:, :].rearrange("p (ch c) -> p ch c", c=3)
            nc.vector.tensor_tensor(out=e3v, in0=e2v, in1=ebv, op=MUL)
            e4 = wk.tile([P, CH * 3], f32)
            e4v = e4[:, :].rearrange("p (ch c) -> p ch c", c=3)
            nc.vector.tensor_tensor(out=e4v, in0=e2v, in1=e2v, op=MUL)

            outt = wk.tile([P, NF * CH * 9], f32)
            ov = outt[:, :].rearrange("p (f ch cs cr) -> p f ch cs cr", f=NF,
                                      cs=3, cr=3)
            nu_of_f = [1, 2, 2, 2, 3, 3, 4, 4, 4, 4, 4]
            epows = {1: ebv, 2: e2v, 3: e3v, 4: e4v}
            # DMA per f-range once the range's expansion ops complete
            dma_ranges = [(0, 4), (4, 8), (8, 11)]
            for f0, f1 in dma_ranges:
                for f in range(f0, f1):
                    eng = nc.gpsimd if f in (9, 10) else nc.vector
                    eng.tensor_tensor(
                        out=ov[:, f, :, :, :],
                        in0=Fv[:, :, f, :].unsqueeze(3).to_broadcast(
                            [P, CH, 3, 3]),
                        in1=epows[nu_of_f[f]].unsqueeze(2).to_broadcast(
                            [P, CH, 3, 3]),
                        op=MUL)
                nc.sync.dma_start(
                    out=out_d[:, f0 * CH * 9:f1 * CH * 9],
                    in_=outt[:, f0 * CH * 9:f1 * CH * 9])
            if debug:
                nc.sync.dma_start(out=dbg["rhs"][:, :], in_=rhs[:, :])
                nc.sync.dma_start(out=dbg["lhsT"][:, :], in_=lhsT[:, :])
                for nm, src in [("A", A), ("Q", Q), ("Ft", Ft)]:
                    nc.sync.dma_start(out=dbg[nm][:, :], in_=src[:, :])
    nc.compile()
    return nc, None


# ---------------- host side -------------------------------------------------
def _host_prep(inputs):
    pos = np.ascontiguousarray(inputs['positions'], np.float32)
    W = np.asarray(inputs['W_embed'], np.float32)
    an = np.asarray(inputs['atomic_numbers'])
    ei = np.asarray(inputs['edge_index'])
    shifts = np.asarray(inputs.get('shifts'), np.float32)
    zs = np.asarray(ZS, an.dtype)
    onehot = (an[:, None] == zs[None, :]).astype(np.float32)
    emb = onehot @ W
    send, recv = ei[0], ei[1]
    order = np.argsort(recv, kind='stable')
    send_s, recv_s = send[order], recv[order]
    vec_all = pos[recv_s] - pos[send_s] + shifts[order]
    embS_all = emb[send_s]
    l_all = np.linalg.norm(vec_all, axis=-1)
    unit_all = vec_all / (l_all + EPS)[:, None]
    n8 = np.arange(1, N_RBF + 1, dtype=np.float32)
    rad_all = (np.sqrt(2.0 / CUTOFF)
               * np.sin(n8 * np.pi * l_all[:, None] / CUTOFF)
               / (l_all + EPS)[:, None])
    u_ = l_all / CUTOFF
    fc = np.where(l_all < CUTOFF,
                  1.0 - 28.0 * u_**6 + 48.0 * u_**7 - 21.0 * u_**8,
                  0.0).astype(np.float32)
    rad_all = rad_all * fc[:, None]
    counts = np.bincount(recv_s, minlength=N_NODES)
    starts = np.concatenate([[0], np.cumsum(counts)])
    in_maps = []
    chunk_meta = []
    for core in range(N_CORES):
        n0, n1 = core * PER, (core + 1) * PER
        chunks = []
        node = n0
        while node < n1:
            base = node
            e_lo = starts[node]
            while (node < n1 and node - base < NT
                   and starts[node + 1] - e_lo <= P):
                node += 1
            assert node > base, f"node {base} degree > {P}"
            chunks.append((int(e_lo), int(starts[node]), int(base)))
        assert len(chunks) <= N_CH, f"core {core}: {len(chunks)} > {N_CH}"
        ed = np.zeros((P, N_CH, 6), np.float32)
        lh = np.zeros((P, N_CH, NT, N_RBF), np.float32)
        eb = np.zeros((NT, N_CH, 3), np.float32)
        for ci, (lo, hi, base) in enumerate(chunks):
            k = hi - lo
            ed[:k, ci, 0:3] = unit_all[lo:hi]
            ed[:k, ci, 3:6] = embS_all[lo:hi]
            rl = recv_s[lo:hi] - base
            lh[np.arange(k), ci, rl, :] = rad_all[lo:hi]
            hi_n = min(base + NT, n1)
            eb[:hi_n - base, ci, :] = emb[base:hi_n]
        ebp = np.repeat(eb, N_RBF, axis=0)   # partition p = n*8 + r
        in_maps.append({
            "ed": np.ascontiguousarray(ed.reshape(P, N_CH * 6)),
            "lh": np.ascontiguousarray(lh.reshape(P, N_CH * NQ)),
            "ebr": np.ascontiguousarray(ebp.reshape(P, N_CH * 3)),
        })
        chunk_meta.append(chunks)
    return in_maps, chunk_meta


_NC_CACHE = [None]
_IN_MAPS_CACHE = [None]


def kernel(**inputs):
    from concourse.bass_utils import run_bass_kernel_spmd
    nc, _ = _build_nc()
    in_maps, chunk_meta = _host_prep(inputs)
    _NC_CACHE[0] = nc
    _IN_MAPS_CACHE[0] = in_maps
    res = run_bass_kernel_spmd(nc, in_maps, core_ids=list(range(N_CORES)))
    out = np.zeros((N_NODES, N_RBF, NF, 9), np.float32)
    for core in range(N_CORES):
        slab = res.results[core]["out"].reshape(NT, N_RBF, NF, N_CH, 9)
        n0, n1 = core * PER, (core + 1) * PER
        chunks = chunk_meta[core]
        for ci, (lo, hi, base) in enumerate(chunks):
            nxt = chunks[ci + 1][2] if ci + 1 < len(chunks) else n1
            out[base:nxt] = slab[:nxt - base, :, :, ci].transpose(0, 1, 2, 3)
    return out
